# revision 1
# baseline (speedup 1.0000x reference)
import math

import ml_dtypes
import numpy as np

B, S, D, H = 2, 2048, 1024, 16
DH = D // H
NCORES = 8
MH = 2 * DH
BS = B * S
PD = D // 128
QT = 512
NQT = S // QT
SCALE = 1.0 / math.sqrt(DH)
BF16 = ml_dtypes.bfloat16

_NC_CACHE = {}


def _build_nc(PKc=9):
    if PKc in _NC_CACHE:
        return _NC_CACHE[PKc]

    from contextlib import ExitStack

    import concourse.bacc as bacc
    import concourse.mybir as mybir
    import concourse.tile as tile

    f32 = mybir.dt.float32
    bf16 = mybir.dt.bfloat16
    Exp = mybir.ActivationFunctionType.Exp

    SVP = PKc * 128
    KT_TILES = []
    off = 0
    while off < SVP:
        w = min(QT, SVP - off)
        KT_TILES.append((off, w))
        off += w
    VG = [(g, min(4, PKc - g)) for g in range(0, PKc, 4)]

    nc = bacc.Bacc("TRN2", target_bir_lowering=False, debug=False)

    xqT = nc.dram_tensor("xqT", [D, BS], bf16, kind="ExternalInput").ap()
    xkT = nc.dram_tensor("xkT", [D, B * SVP], bf16, kind="ExternalInput").ap()
    xvT = nc.dram_tensor("xvT", [D, B * SVP], bf16, kind="ExternalInput").ap()
    wq = nc.dram_tensor("wq", [128, PD, MH], bf16, kind="ExternalInput").ap()
    wk = nc.dram_tensor("wk", [128, PD, MH], bf16, kind="ExternalInput").ap()
    wv = nc.dram_tensor("wv", [128, PD, MH], bf16, kind="ExternalInput").ap()
    wo = nc.dram_tensor("wo", [128, D], bf16, kind="ExternalInput").ap()
    mb = nc.dram_tensor("mb", [128, B, PKc], f32, kind="ExternalInput").ap()
    out = nc.dram_tensor("out", [BS, D], bf16, kind="ExternalOutput").ap()
    import os
    DBG = bool(int(os.environ.get("K2_DEBUG", "0")))
    NOZF = bool(int(os.environ.get("K2_NOZFOLD", "0")))
    if DBG:
        dbg_q = nc.dram_tensor("dbg_q", [128, BS], bf16, kind="ExternalOutput").ap()
        dbg_k = nc.dram_tensor("dbg_k", [128, B * SVP], bf16, kind="ExternalOutput").ap()
        dbg_va = nc.dram_tensor("dbg_va", [128, B, PKc, 4, 64], bf16, kind="ExternalOutput").ap()
        dbg_otn = nc.dram_tensor("dbg_otn", [128, B, S], bf16, kind="ExternalOutput").ap()

    with tile.TileContext(nc) as tc, ExitStack() as ctx:
        wpool = ctx.enter_context(tc.tile_pool(name="wpool", bufs=1))
        apool = ctx.enter_context(tc.tile_pool(name="apool", bufs=1))

        wq_sb = wpool.tile([128, PD, MH], bf16)
        wk_sb = wpool.tile([128, PD, MH], bf16)
        wv_sb = wpool.tile([128, PD, MH], bf16)
        wo_sb = wpool.tile([128, D], bf16)
        mb_sb = wpool.tile([128, B, PKc], f32)
        nc.sync.dma_start(wk_sb, wk)
        nc.sync.dma_start(mb_sb, mb)

        scratch = wpool.tile([1, 2], f32)
        nc.vector.tensor_copy(scratch, mb_sb[0:1, 0, 0:2])
        scratch2 = wpool.tile([1, 2], f32)
        nc.scalar.activation(scratch2, mb_sb[0:1, 0, 0:2], Exp)

        qT_sb = apool.tile([128, BS], bf16)
        kT_sb = apool.tile([128, B * SVP], bf16)
        va_sb = apool.tile([128, B, PKc, 4, 64], bf16)
        nc.vector.memset(va_sb, 0.0)
        nc.vector.memset(va_sb[:, :, :, 1, 0:1], 1.0)
        nc.vector.memset(va_sb[:, :, :, 2, 0:1], 1.0)
        ones_sb = apool.tile([128, 64], bf16)
        nc.vector.memset(ones_sb, 1.0)

        xhp = ctx.enter_context(tc.tile_pool(name="xhp", bufs=1))
        with (
            tc.tile_pool(name="atp", bufs=12) as atp,
            tc.tile_pool(name="rp", bufs=2) as rp,
            tc.tile_pool(name="op", bufs=2) as op,
            tc.tile_pool(name="outp", bufs=6) as outp,
            tc.tile_pool(name="psp", bufs=2, space="PSUM") as psp,
            tc.tile_pool(name="pss", bufs=2, space="PSUM") as pss,
            tc.tile_pool(name="pab", bufs=1, space="PSUM") as pab,
        ):
            ws_count = [0]
            pending_wo = []

            def emit_wo():
                b, qt, otn = pending_wo.pop()
                for st in range(qt * 4, qt * 4 + 4):
                    rs = b * S + st * 128
                    ws = outp.tile([128, D], bf16, tag="ws")
                    for nt in range(2):
                        wp = psp.tile([128, QT], f32, tag="pq")
                        nc.tensor.matmul(
                            wp,
                            lhsT=otn[:, st * 128:(st + 1) * 128],
                            rhs=wo_sb[:, nt * QT:(nt + 1) * QT],
                        )
                        nc.vector.tensor_copy(ws[:, nt * QT:(nt + 1) * QT], wp)
                    nc.scalar.dma_start(out[rs:rs + 128, :], ws)

            def q_proj(b, sti):
                pq = psp.tile([128, QT], f32, tag="pq")
                for kc in range(PD):
                    nc.tensor.matmul(
                        pq,
                        lhsT=wq_sb[:, kc, :],
                        rhs=xh_q[:, kc, sti * QT:(sti + 1) * QT],
                        start=(kc == 0),
                        stop=(kc == PD - 1),
                    )
                ds = b * S + sti * QT
                nc.vector.tensor_copy(qT_sb[:, ds:ds + QT], pq)

            def attn_mm(psA, psB, attn, b, kc, zrep=False):
                if NOZF or zrep:
                    nc.tensor.matmul(
                        psA[0:64, :], lhsT=va_sb[:, b, kc, 0, :],
                        rhs=attn[:, 0, :],
                        start=(kc == 0), stop=(kc == PKc - 1),
                        skip_group_check=True,
                    )
                    nc.tensor.matmul(
                        psA[64:128, :], lhsT=va_sb[:, b, kc, 3, :],
                        rhs=attn[:, 1, :],
                        start=(kc == 0), stop=(kc == PKc - 1),
                        skip_group_check=True,
                    )
                    nc.tensor.matmul(
                        psB[0:64, :], lhsT=ones_sb[:, 0:64], rhs=attn[:, 0, :],
                        start=(kc == 0), stop=(kc == PKc - 1),
                        skip_group_check=True,
                    )
                    nc.tensor.matmul(
                        psB[64:128, :], lhsT=ones_sb[:, 0:64], rhs=attn[:, 1, :],
                        start=(kc == 0), stop=(kc == PKc - 1),
                        skip_group_check=True,
                    )
                    return
                nc.tensor.matmul(
                    psA, lhsT=va_sb[:, b, kc, 0:2, :], rhs=attn[:, 0, :],
                    start=(kc == 0), stop=(kc == PKc - 1),
                    skip_group_check=True,
                )
                nc.tensor.matmul(
                    psB, lhsT=va_sb[:, b, kc, 2:4, :], rhs=attn[:, 1, :],
                    start=(kc == 0), stop=(kc == PKc - 1),
                    skip_group_check=True,
                )

            for b in range(B):
                xh_k = xhp.tile([128, PD, SVP], bf16, tag="xk")
                xh_v = xhp.tile([128, PD, SVP], bf16, tag="xv")
                xh_q = xhp.tile([128, PD, S], bf16, tag="xq")
                for kc in range(PD):
                    nc.sync.dma_start(
                        xh_k[:, kc, :],
                        xkT[kc * 128:(kc + 1) * 128, b * SVP:(b + 1) * SVP],
                    )
                if b == 0:
                    nc.sync.dma_start(wq_sb, wq)
                chunks = ([(0, 512), "v", (512, 1024), (1024, 2048)]
                          if b == 0 else ["v", (0, 1024), (1024, 2048)])
                for item in chunks:
                    if item == "v":
                        if b == 0:
                            nc.sync.dma_start(wv_sb, wv)
                        for kc in range(PD):
                            nc.sync.dma_start(
                                xh_v[:, kc, :],
                                xvT[kc * 128:(kc + 1) * 128,
                                    b * SVP:(b + 1) * SVP],
                            )
                        if b == 0:
                            nc.sync.dma_start(wo_sb, wo)
                        continue
                    c0, c1 = item
                    for kc in range(PD):
                        nc.sync.dma_start(
                            xh_q[:, kc, c0:c1],
                            xqT[kc * 128:(kc + 1) * 128, b * S + c0:b * S + c1],
                        )
                for (toff, tw) in KT_TILES:
                    pk = psp.tile([128, QT], f32, tag="pq")
                    for kc in range(PD):
                        nc.tensor.matmul(
                            pk[:, 0:tw],
                            lhsT=wk_sb[:, kc, :],
                            rhs=xh_k[:, kc, toff:toff + tw],
                            start=(kc == 0),
                            stop=(kc == PD - 1),
                        )
                    ds = b * SVP + toff
                    nc.vector.tensor_copy(kT_sb[:, ds:ds + tw], pk[:, 0:tw])
                def v_proj():
                    for (g0, gn) in VG:
                        pv = psp.tile([128, QT], f32, tag="pq")
                        for j in range(gn):
                            for dc in range(PD):
                                nc.tensor.matmul(
                                    pv[:, j * 128:(j + 1) * 128],
                                    lhsT=xh_v[:, dc,
                                              (g0 + j) * 128:(g0 + j + 1) * 128],
                                    rhs=wv_sb[:, dc, :],
                                    start=(dc == 0),
                                    stop=(dc == PD - 1),
                                    skip_group_check=True,
                                )
                        nc.vector.tensor_copy(
                            va_sb[:, b, g0:g0 + gn, ::3, :],
                            pv[:, 0:gn * 128],
                        )

                if b > 0:
                    v_proj()
                q_proj(b, 0)
                otn = op.tile([128, S], bf16, tag="otn")
                for qt in range(NQT):
                    qs = b * S + qt * QT
                    last_qt = (b == B - 1 and qt == NQT - 1) and not NOZF
                    psA = pab.tile([128, QT], f32, tag="psA")
                    psB = pab.tile([128, QT], f32, tag="psB")
                    attns = []
                    for kc in range(PKc):
                        ks = b * SVP + kc * 128
                        sc = pss.tile([128, 2, QT], f32, tag="sc")
                        nc.tensor.matmul(
                            sc[:, 0, :],
                            lhsT=kT_sb[0:DH, ks:ks + 128],
                            rhs=qT_sb[0:DH, qs:qs + QT],
                        )
                        nc.tensor.matmul(
                            sc[:, 1, :],
                            lhsT=kT_sb[DH:128, ks:ks + 128],
                            rhs=qT_sb[DH:128, qs:qs + QT],
                        )
                        attn = atp.tile([128, 2, QT], bf16, tag="attn")
                        nc.scalar.activation(attn, sc, Exp, scale=SCALE,
                                             bias=mb_sb[:, b, kc:kc + 1])
                        attns.append(attn)
                        if kc == 3 and pending_wo:
                            emit_wo()
                        if kc >= 4 and not (b == 0 and qt == 0):
                            attn_mm(psA, psB, attns[kc - 4], b, kc - 4,
                                    zrep=last_qt)
                    if b == 0 and qt == 0:
                        v_proj()
                        for kc in range(PKc - 4):
                            attn_mm(psA, psB, attns[kc], b, kc)
                    for kc in range(max(0, PKc - 4), PKc):
                        attn_mm(psA, psB, attns[kc], b, kc, zrep=last_qt)

                    rz = rp.tile([128, QT], f32, tag="rz")
                    rzb = rp.tile([128, QT], bf16, tag="rzb")
                    if last_qt:
                        if pending_wo:
                            emit_wo()

                        def tail_norm(st):
                            j = (st - qt * 4) * 128
                            nc.vector.reciprocal_approx_fast(
                                rz[:, j:j + 128], psB[:, j:j + 128])
                            nc.vector.tensor_mul(
                                otn[:, st * 128:(st + 1) * 128],
                                psA[:, j:j + 128], rz[:, j:j + 128]
                            )

                        def tail_wo(st):
                            rs = b * S + st * 128
                            ws = outp.tile([128, D], bf16, tag="ws")
                            for nt in range(2):
                                wp = psp.tile([128, QT], f32, tag="pq")
                                nc.tensor.matmul(
                                    wp,
                                    lhsT=otn[:, st * 128:(st + 1) * 128],
                                    rhs=wo_sb[:, nt * QT:(nt + 1) * QT],
                                )
                                nc.scalar.copy(
                                    ws[:, nt * QT:(nt + 1) * QT], wp)
                            nc.scalar.dma_start(out[rs:rs + 128, :], ws)

                        tail_norm(qt * 4)
                        for st in range(qt * 4, qt * 4 + 4):
                            if st + 1 < qt * 4 + 4:
                                tail_norm(st + 1)
                            tail_wo(st)
                        continue
                    if NOZF:
                        nc.vector.reciprocal_approx_fast(rz, psB)
                        nc.vector.tensor_mul(
                            otn[:, qt * QT:(qt + 1) * QT], psA, rz)
                        if pending_wo:
                            emit_wo()
                        if qt + 1 < NQT:
                            q_proj(b, qt + 1)
                        pending_wo.append((b, qt, otn))
                        continue
                    rz2 = rp.tile([128, QT], f32, tag="rz2")
                    nc.vector.reciprocal_approx_fast(rz, psA)
                    nc.vector.reciprocal_approx_fast(rz2, psB)
                    nc.vector.tensor_copy(rzb[64:65, :], rz[64:65, :])
                    nc.vector.tensor_copy(rzb[0:1, :], rz2[0:1, :])
                    nc.tensor.matmul(
                        psB[0:64, :], lhsT=ones_sb[64:65, :], rhs=rzb[64:65, :],
                        skip_group_check=True,
                    )
                    nc.tensor.matmul(
                        psA[64:128, :], lhsT=ones_sb[0:1, :], rhs=rzb[0:1, :],
                        skip_group_check=True,
                    )
                    rb = rp.tile([128, QT], f32, tag="rb")
                    nc.vector.tensor_copy(rb[0:64, :], psB[0:64, :])
                    nc.vector.tensor_copy(rb[64:128, :], psA[64:128, :])
                    nc.vector.tensor_mul(
                        otn[0:64, qt * QT:(qt + 1) * QT], psA[0:64, :],
                        rb[0:64, :]
                    )
                    nc.vector.tensor_mul(
                        otn[64:128, qt * QT:(qt + 1) * QT], psB[64:128, :],
                        rb[64:128, :]
                    )

                    if qt + 1 < NQT:
                        q_proj(b, qt + 1)
                    pending_wo.append((b, qt, otn))

    nc.compile()
    _NC_CACHE[PKc] = nc
    return nc


def _prep_inputs(queries, keys, values, masks, Wq, Wk, Wv, Wo):
    def t_bf16(x):
        return np.ascontiguousarray(
            np.asarray(x, dtype=np.float32).reshape(-1, D).astype(BF16).T
        )

    m01 = np.asarray(masks) != 0
    sv = m01.sum(axis=1)
    PKc = max(1, int(-(-int(sv.max()) // 128)))
    SVP = PKc * 128

    keys_f = np.asarray(keys, dtype=np.float32)
    vals_f = np.asarray(values, dtype=np.float32)
    kc_ = np.zeros((B, SVP, D), dtype=np.float32)
    vc_ = np.zeros((B, SVP, D), dtype=np.float32)
    mbias = np.full((B, SVP), -30000.0, dtype=np.float32)
    for b in range(B):
        idx = np.nonzero(m01[b])[0]
        kc_[b, :len(idx)] = keys_f[b, idx]
        vc_[b, :len(idx)] = vals_f[b, idx]
        mbias[b, :len(idx)] = 0.0

    xqT = t_bf16(queries)
    xkT = np.ascontiguousarray(kc_.reshape(B * SVP, D).astype(BF16).T)
    xvT = np.ascontiguousarray(vc_.reshape(B * SVP, D).astype(BF16).T)
    mb = np.ascontiguousarray(
        mbias.reshape(B, PKc, 128).transpose(2, 0, 1).astype(np.float32)
    )

    def w_prep(W, c):
        Wc = np.asarray(W, dtype=np.float32)[:, c * MH:(c + 1) * MH]
        return np.ascontiguousarray(
            Wc.astype(BF16).reshape(PD, 128, MH).transpose(1, 0, 2)
        )

    Wo_f = np.asarray(Wo, dtype=np.float32)
    in_maps = []
    for c in range(NCORES):
        in_maps.append({
            "xqT": xqT, "xkT": xkT, "xvT": xvT,
            "wq": w_prep(Wq, c), "wk": w_prep(Wk, c), "wv": w_prep(Wv, c),
            "wo": np.ascontiguousarray(
                Wo_f[c * MH:(c + 1) * MH, :].astype(BF16)
            ),
            "mb": mb,
        })
    return PKc, in_maps


def run(inputs, trace=False, trace_cores=None):
    from concourse.bass_utils import run_bass_kernel_spmd

    PKc, in_maps = _prep_inputs(**inputs)
    nc = _build_nc(PKc)
    res = run_bass_kernel_spmd(
        nc, in_maps, core_ids=list(range(NCORES)),
        trace=trace, trace_cores=trace_cores,
    )
    acc = res.results[0]["out"].astype(np.float32)
    for r in res.results[1:]:
        acc += r["out"].astype(np.float32)
    return acc.reshape(B, S, D), res


def kernel(**inputs) -> np.ndarray:
    out, _ = run(inputs)
    return out

